# revision 38
# baseline (speedup 1.0000x reference)
"""Trainium2 Bass kernel for batched attention with softmax over the query axis.

Reference computation (per batch element b):
    Q = tokens @ Wq; K = tokens @ Wk; V = tokens @ Wv
    S = Q @ K.T                [T(t), T(s)]
    A = softmax(S, axis=t)     (normalizes over the *query* axis per key column)
    out = A @ V                [T, H]

Sharding: pure data parallelism — B=8 batch elements, one per NeuronCore.
The softmax couples queries only within a batch element, so no collectives.

Per-core implementation (fp16 matmul operands, fp32 PSUM accumulation;
validated numerically at ~0.26% relative error):
  - W_qk = Wq @ Wk.T is built on-chip (weight-only work that overlaps the
    token DMA), so scores need one projection G = tokens @ W_qk instead of
    separate Q and K: S = G @ tokens.T.
  - tokT [h%128, h//128, t] = tokens.T via f32 PE transpose straight off the
    DMA stage (the PSUM->SBUF evacuation doubles as the fp16 cast).
  - GT   [g%128, g//128, t] via lhsT=W_qk rhs=tokT.
  - V    [s%128, s//128, h] via lhsT=tokT rhs=Wv.
  - S_st [s%128, t] via lhsT=tokT rhs=GT -> softmax over t is a free-axis
    reduction: max (DVE), exp (ScalarE, accum_out produces row sums).
  - 1/rowsum is folded into V rows (GpSimd), so the unnormalized exp tile E
    feeds the context matmul: ctx[t,h] via lhsT=E rhs=V' accumulated over s.
Engine balance: GpSimd does f32->f16 casts and V scaling, DVE does PSUM
evacuations and reductions, ScalarE does exp and the GT evacuations.
"""

import numpy as np

import concourse.bass as bass
import concourse.bacc as bacc
import concourse.tile as tile
from concourse import mybir
from concourse.bass_utils import run_bass_kernel_spmd
from concourse.masks import make_identity

B, T, H, E = 8, 2048, 512, 512
P = 128
NT = T // P      # 16 tiles along t / s
NH = H // P      # 4 tiles along h
FD = 512         # matmul moving free dim (one fp32 PSUM bank)
NC_T = T // FD   # 4 free-dim chunks along t
NST = T // FD    # 4 token stage groups (4 t-tiles each)

F32 = mybir.dt.float32
F16 = mybir.dt.float16
AX = mybir.AxisListType
AF = mybir.ActivationFunctionType

N_CORES = 8


def build():
    nc = bacc.Bacc()
    tok_d = nc.declare_dram_parameter("tokens", [T, H], F32, isOutput=False)
    wq_d = nc.declare_dram_parameter("Wq", [H, E], F32, isOutput=False)
    wk_d = nc.declare_dram_parameter("Wk", [H, E], F32, isOutput=False)
    wv_d = nc.declare_dram_parameter("Wv", [H, H], F32, isOutput=False)
    out_d = nc.declare_dram_parameter("out", [T, H], F32, isOutput=True)

    # [p, tt, h]: partition = t%128, stage groups of 4 t-tiles -> 1MB DMAs
    tok_staged = tok_d.rearrange("(sg tt p) h -> sg p tt h", p=P, tt=NT // NST)
    out_tiled = out_d.rearrange("(tt p) h -> tt p h", p=P)

    with tile.TileContext(nc) as tc:
        with (
            tc.tile_pool(name="persist", bufs=1) as pp,
            tc.tile_pool(name="stage", bufs=2) as sp,
            tc.tile_pool(name="ostage", bufs=3) as osp,
            tc.tile_pool(name="stats", bufs=4) as stp,
            tc.tile_pool(name="psum", bufs=8, space=bass.MemorySpace.PSUM) as psp,
        ):
            ident32 = pp.tile([P, P], F32, tag="ident32")
            make_identity(nc, ident32[:])
            ident = pp.tile([P, P], F16, tag="ident")
            make_identity(nc, ident[:])

            # Minimal HAM warm-up: ~1.8us of dense dummy transposes bridge the
            # sparse DMA-paced W-transpose window so the activity monitor's
            # SHORT window fires early and the real matmuls run at 2.4 GHz.
            for wi in range(12):
                ps_w = psp.tile([P, P], F16, tag="mm", name=f"warm{wi}")
                nc.tensor.transpose(ps_w[:], ident[:], ident[:])

            # ---- Wq/Wk: chunked loads, f32 PE transpose straight off the
            # stage (first PE work starts ~1.5us in), evac to fp16 WqT/WkT.
            wT16 = {}
            for name, wd in (("wq", wq_d), ("wk", wk_d)):
                wT = pp.tile([P, NH, E], F16, tag=f"{name}T", name=f"wT_{name}")
                wtiled = wd.rearrange("(hh p) e -> hh p e", p=P)
                for hh in range(NH):
                    wstage = sp.tile([P, E], F32, tag=f"wstage_{name}",
                                     bufs=4, name=f"wst_{name}{hh}")
                    nc.sync.dma_start(wstage[:], wtiled[hh])
                    for eb in range(NH):
                        ps_tr = psp.tile([P, P], F32, tag="mm",
                                         name=f"tr_{name}{hh}{eb}")
                        nc.tensor.transpose(
                            ps_tr[:], wstage[:, eb * P : (eb + 1) * P], ident32[:]
                        )
                        nc.vector.tensor_copy(
                            wT[:, eb, hh * P : (hh + 1) * P], ps_tr[:]
                        )
                wT16[name] = wT

            # ---- W_qk = Wq @ Wk.T : [h%128, hb, h'] fp16 ----
            Wqk = pp.tile([P, NH, H], F16, tag="Wqk")
            for hb in range(NH):
                ps = psp.tile([P, FD], F32, tag="mm", name=f"ps_wqk{hb}")
                for eb in range(NH):
                    nc.tensor.matmul(
                        ps[:],
                        wT16["wq"][:, eb, hb * P : (hb + 1) * P],
                        wT16["wk"][:, eb, :],
                        start=(eb == 0),
                        stop=(eb == NH - 1),
                    )
                nc.vector.tensor_copy(Wqk[:, hb, :], ps[:])

            # ---- tokens: 1MB staged DMAs; f32 PE transpose straight off the
            # stage (the PSUM->SBUF evac is the fp16 cast). Per stage group,
            # emit in data-arrival order: transposes -> GT chunk -> V tiles,
            # so the static PE stream never blocks on not-yet-landed data.
            tokT = pp.tile([P, NH, T], F16, tag="tokT")
            tstages = []
            for sg in range(NST):
                tstage = sp.tile([P, NT // NST, H], F32, tag="tstage", bufs=4,
                                 name=f"tst{sg}")
                tstages.append(tstage)

            for ti in range(NT // NST):
                nc.sync.dma_start(tstages[0][:, ti], tok_staged[0][:, ti])

            # Wv after token stage 0: load f32, cast fp16 (GpSimd, idle).
            wv_stage = sp.tile([P, NH, E], F32, tag="wvstage", bufs=1)
            nc.sync.dma_start(wv_stage[:], wv_d.rearrange("(hh p) e -> p hh e", p=P))
            wv16 = pp.tile([P, NH, E], F16, tag="wv16")
            for hh in range(NH):
                nc.gpsimd.tensor_copy(wv16[:, hh], wv_stage[:, hh])

            for sg in range(1, NST):
                nc.sync.dma_start(tstages[sg][:], tok_staged[sg])

            def emit_transposes(sg):
                for ti in range(NT // NST):
                    tt = sg * (NT // NST) + ti
                    t16 = sp.tile([P, H], F16, tag="t16", bufs=4, name=f"t16_{tt}")
                    nc.vector.tensor_copy(t16[:], tstages[sg][:, ti])
                    ps_tr = psp.tile([P, NH, P], F16, tag="mm", name=f"trt{tt}")
                    for ht in range(NH):
                        nc.tensor.transpose(
                            ps_tr[:, ht],
                            t16[:, ht * P : (ht + 1) * P],
                            ident[:],
                        )
                    nc.vector.tensor_copy(
                        tokT[:, :, tt * P : (tt + 1) * P], ps_tr[:]
                    )

            # ---- per stage: transposes -> GT chunk -> V tiles ----
            GT = pp.tile([P, NH, T], F16, tag="GT")
            V = pp.tile([P, NT, H], F16, tag="V")
            for sg in range(NST):
                emit_transposes(sg)
                tch = sg
                for gb in range(NH):
                    ps = psp.tile([P, FD], F32, tag="mm", name=f"ps_g{gb}_{tch}")
                    for hb in range(NH):
                        nc.tensor.matmul(
                            ps[:],
                            Wqk[:, hb, gb * P : (gb + 1) * P],
                            tokT[:, hb, tch * FD : (tch + 1) * FD],
                            start=(hb == 0),
                            stop=(hb == NH - 1),
                        )
                    nc.scalar.copy(GT[:, gb, tch * FD : (tch + 1) * FD], ps[:])
                for st in range(sg * NC_T, (sg + 1) * NC_T):
                    ps = psp.tile([P, FD], F32, tag="mm", name=f"ps_v{st}")
                    for ht in range(NH):
                        nc.tensor.matmul(
                            ps[:],
                            tokT[:, ht, st * P : (st + 1) * P],
                            wv16[:, ht, :],
                            start=(ht == 0),
                            stop=(ht == NH - 1),
                        )
                    nc.vector.tensor_copy(V[:, st, :], ps[:])

            # ---- scores S[s,t] + softmax over t (free axis) ----
            Etile = pp.tile([P, NT, T], F16, tag="E")
            for st in range(NT):
                pss = [
                    psp.tile([P, FD], F32, tag="mm", name=f"ps_s{st}_{tch}")
                    for tch in range(NC_T)
                ]
                for tch in range(NC_T):
                    for hb in range(NH):
                        nc.tensor.matmul(
                            pss[tch][:],
                            tokT[:, hb, st * P : (st + 1) * P],
                            GT[:, hb, tch * FD : (tch + 1) * FD],
                            start=(hb == 0),
                            stop=(hb == NH - 1),
                        )
                mx4 = stp.tile([P, NC_T], F32, tag="mx4")
                for tch in range(NC_T):
                    nc.vector.reduce_max(
                        mx4[:, tch : tch + 1], pss[tch][:], axis=AX.X
                    )
                nmx = stp.tile([P, 1], F32, tag="nmx")
                nc.vector.reduce_max(nmx[:], mx4[:], axis=AX.X, negate=True)
                racc = stp.tile([P, NC_T], F32, tag="racc")
                for tch in range(NC_T):
                    nc.scalar.activation(
                        Etile[:, st, tch * FD : (tch + 1) * FD],
                        pss[tch][:],
                        AF.Exp,
                        bias=nmx[:],
                        accum_out=racc[:, tch : tch + 1],
                    )
                rsum = stp.tile([P, 1], F32, tag="rsum")
                nc.vector.reduce_sum(rsum[:], racc[:], axis=AX.X)
                rinv = stp.tile([P, 1], F32, tag="rinv")
                nc.vector.reciprocal(rinv[:], rsum[:])
                # Fold 1/rowsum into V rows (rowsum is per-s, V is s-major).
                nc.vector.tensor_scalar_mul(V[:, st, :], V[:, st, :], rinv[:])

            # ---- context: ctx[t,h] = sum_s E[s,t] * V'[s,h] ----
            for tt in range(NT):
                ps = psp.tile([P, FD], F32, tag="mm", name=f"ps_c{tt}")
                for st in range(NT):
                    nc.tensor.matmul(
                        ps[:],
                        Etile[:, st, tt * P : (tt + 1) * P],
                        V[:, st, :],
                        start=(st == 0),
                        stop=(st == NT - 1),
                    )
                ot = osp.tile([P, H], F32, tag="ostage", name=f"ost{tt}")
                if tt >= NT - 2:
                    for cc in range(2):
                        sl = slice(cc * (H // 2), (cc + 1) * (H // 2))
                        nc.vector.tensor_copy(ot[:, sl], ps[:, sl])
                        nc.sync.dma_start(out_tiled[tt][:, sl], ot[:, sl])
                else:
                    nc.vector.tensor_copy(ot[:], ps[:])
                    nc.sync.dma_start(out_tiled[tt], ot[:])

    nc.compile()
    return nc


_NC = None


def _get_nc():
    global _NC
    if _NC is None:
        _NC = build()
    return _NC


def _run(inputs, trace=False, **kwargs):
    tokens = np.ascontiguousarray(inputs["tokens"], dtype=np.float32)
    Wq = np.ascontiguousarray(inputs["Wq"], dtype=np.float32)
    Wk = np.ascontiguousarray(inputs["Wk"], dtype=np.float32)
    Wv = np.ascontiguousarray(inputs["Wv"], dtype=np.float32)
    assert tokens.shape == (B, T, H)
    nc = _get_nc()
    in_maps = [
        {"tokens": tokens[i], "Wq": Wq, "Wk": Wk, "Wv": Wv} for i in range(N_CORES)
    ]
    res = run_bass_kernel_spmd(
        nc, in_maps, core_ids=list(range(N_CORES)), trace=trace, **kwargs
    )
    out = np.stack([res.results[i]["out"] for i in range(N_CORES)], axis=0)
    return out.astype(np.float32), res


def kernel(**inputs) -> np.ndarray:
    out, _ = _run(inputs)
    return out


# revision 39
# speedup vs baseline: 1.1993x; 1.1993x over previous
"""Trainium2 Bass kernel for batched attention with softmax over the query axis.

Reference computation (per batch element b):
    Q = tokens @ Wq; K = tokens @ Wk; V = tokens @ Wv
    S = Q @ K.T                [T(t), T(s)]
    A = softmax(S, axis=t)     (normalizes over the *query* axis per key column)
    out = A @ V                [T, H]

Sharding: pure data parallelism — B=8 batch elements, one per NeuronCore.
The softmax couples queries only within a batch element, so no collectives.

Per-core implementation (fp16 matmul operands, fp32 PSUM accumulation;
validated numerically at ~0.26% relative error):
  - W_qk = Wq @ Wk.T is built on-chip (weight-only work that overlaps the
    token DMA), so scores need one projection G = tokens @ W_qk instead of
    separate Q and K: S = G @ tokens.T.
  - tokT [h%128, h//128, t] = tokens.T via f32 PE transpose straight off the
    DMA stage (the PSUM->SBUF evacuation doubles as the fp16 cast).
  - GT   [g%128, g//128, t] via lhsT=W_qk rhs=tokT.
  - V    [s%128, s//128, h] via lhsT=tokT rhs=Wv.
  - S_st [s%128, t] via lhsT=tokT rhs=GT -> softmax over t is a free-axis
    reduction: max (DVE), exp (ScalarE, accum_out produces row sums).
  - 1/rowsum is folded into V rows (GpSimd), so the unnormalized exp tile E
    feeds the context matmul: ctx[t,h] via lhsT=E rhs=V' accumulated over s.
Engine balance: GpSimd does f32->f16 casts and V scaling, DVE does PSUM
evacuations and reductions, ScalarE does exp and the GT evacuations.
"""

import numpy as np

import concourse.bass as bass
import concourse.bacc as bacc
import concourse.tile as tile
from concourse import mybir
from concourse.bass_utils import run_bass_kernel_spmd
from concourse.masks import make_identity

B, T, H, E = 8, 2048, 512, 512
P = 128
NT = T // P      # 16 tiles along t / s
NH = H // P      # 4 tiles along h
FD = 512         # matmul moving free dim (one fp32 PSUM bank)
NC_T = T // FD   # 4 free-dim chunks along t
NST = T // FD    # 4 token stage groups (4 t-tiles each)

F32 = mybir.dt.float32
F16 = mybir.dt.float16
AX = mybir.AxisListType
AF = mybir.ActivationFunctionType

N_CORES = 8


def build():
    nc = bacc.Bacc()
    tok_d = nc.declare_dram_parameter("tokens", [T, H], F32, isOutput=False)
    wq_d = nc.declare_dram_parameter("Wq", [H, E], F32, isOutput=False)
    wk_d = nc.declare_dram_parameter("Wk", [H, E], F32, isOutput=False)
    wv_d = nc.declare_dram_parameter("Wv", [H, H], F32, isOutput=False)
    out_d = nc.declare_dram_parameter("out", [T, H], F32, isOutput=True)

    # [p, tt, h]: partition = t%128, stage groups of 4 t-tiles -> 1MB DMAs
    tok_staged = tok_d.rearrange("(sg tt p) h -> sg p tt h", p=P, tt=NT // NST)
    out_tiled = out_d.rearrange("(tt p) h -> tt p h", p=P)

    with tile.TileContext(nc) as tc:
        with (
            tc.tile_pool(name="persist", bufs=1) as pp,
            tc.tile_pool(name="stage", bufs=2) as sp,
            tc.tile_pool(name="ostage", bufs=3) as osp,
            tc.tile_pool(name="stats", bufs=4) as stp,
            tc.tile_pool(name="psum", bufs=8, space=bass.MemorySpace.PSUM) as psp,
        ):
            ident32 = pp.tile([P, P], F32, tag="ident32")
            make_identity(nc, ident32[:])
            ident = pp.tile([P, P], F16, tag="ident")
            make_identity(nc, ident[:])

            # ---- Wq/Wk: chunked loads, f32 PE transpose straight off the
            # stage (first PE work starts ~1.5us in), evac to fp16 WqT/WkT.
            wT16 = {}
            for name, wd in (("wq", wq_d), ("wk", wk_d)):
                wT = pp.tile([P, NH, E], F16, tag=f"{name}T", name=f"wT_{name}")
                wtiled = wd.rearrange("(hh p) e -> hh p e", p=P)
                for hh in range(NH):
                    wstage = sp.tile([P, E], F32, tag=f"wstage_{name}",
                                     bufs=4, name=f"wst_{name}{hh}")
                    nc.sync.dma_start(wstage[:], wtiled[hh])
                    for eb in range(NH):
                        ps_tr = psp.tile([P, P], F32, tag="mm",
                                         name=f"tr_{name}{hh}{eb}")
                        nc.tensor.transpose(
                            ps_tr[:], wstage[:, eb * P : (eb + 1) * P], ident32[:]
                        )
                        nc.vector.tensor_copy(
                            wT[:, eb, hh * P : (hh + 1) * P], ps_tr[:]
                        )
                wT16[name] = wT

            # ---- W_qk = Wq @ Wk.T : [h%128, hb, h'] fp16 ----
            Wqk = pp.tile([P, NH, H], F16, tag="Wqk")
            for hb in range(NH):
                ps = psp.tile([P, FD], F32, tag="mm", name=f"ps_wqk{hb}")
                for eb in range(NH):
                    nc.tensor.matmul(
                        ps[:],
                        wT16["wq"][:, eb, hb * P : (hb + 1) * P],
                        wT16["wk"][:, eb, :],
                        start=(eb == 0),
                        stop=(eb == NH - 1),
                    )
                nc.vector.tensor_copy(Wqk[:, hb, :], ps[:])

            # ---- tokens: 1MB staged DMAs; f32 PE transpose straight off the
            # stage (the PSUM->SBUF evac is the fp16 cast). Per stage group,
            # emit in data-arrival order: transposes -> GT chunk -> V tiles,
            # so the static PE stream never blocks on not-yet-landed data.
            tokT = pp.tile([P, NH, T], F16, tag="tokT")
            tstages = []
            for sg in range(NST):
                tstage = sp.tile([P, NT // NST, H], F32, tag="tstage", bufs=4,
                                 name=f"tst{sg}")
                tstages.append(tstage)

            for ti in range(NT // NST):
                nc.sync.dma_start(tstages[0][:, ti], tok_staged[0][:, ti])

            # Wv after token stage 0: load f32, cast fp16 (GpSimd, idle).
            wv_stage = sp.tile([P, NH, E], F32, tag="wvstage", bufs=1)
            nc.sync.dma_start(wv_stage[:], wv_d.rearrange("(hh p) e -> p hh e", p=P))
            wv16 = pp.tile([P, NH, E], F16, tag="wv16")
            for hh in range(NH):
                nc.gpsimd.tensor_copy(wv16[:, hh], wv_stage[:, hh])

            for sg in range(1, NST):
                nc.sync.dma_start(tstages[sg][:], tok_staged[sg])

            def emit_transposes(sg):
                for ti in range(NT // NST):
                    tt = sg * (NT // NST) + ti
                    t16 = sp.tile([P, H], F16, tag="t16", bufs=4, name=f"t16_{tt}")
                    nc.vector.tensor_copy(t16[:], tstages[sg][:, ti])
                    ps_tr = psp.tile([P, NH, P], F16, tag="mm", name=f"trt{tt}")
                    for ht in range(NH):
                        nc.tensor.transpose(
                            ps_tr[:, ht],
                            t16[:, ht * P : (ht + 1) * P],
                            ident[:],
                        )
                    nc.vector.tensor_copy(
                        tokT[:, :, tt * P : (tt + 1) * P], ps_tr[:]
                    )

            # ---- per stage: transposes -> GT chunk -> V tiles ----
            GT = pp.tile([P, NH, T], F16, tag="GT")
            V = pp.tile([P, NT, H], F16, tag="V")
            for sg in range(NST):
                emit_transposes(sg)
                tch = sg
                for gb in range(NH):
                    ps = psp.tile([P, FD], F32, tag="mm", name=f"ps_g{gb}_{tch}")
                    for hb in range(NH):
                        nc.tensor.matmul(
                            ps[:],
                            Wqk[:, hb, gb * P : (gb + 1) * P],
                            tokT[:, hb, tch * FD : (tch + 1) * FD],
                            start=(hb == 0),
                            stop=(hb == NH - 1),
                        )
                    nc.scalar.copy(GT[:, gb, tch * FD : (tch + 1) * FD], ps[:])
                for st in range(sg * NC_T, (sg + 1) * NC_T):
                    ps = psp.tile([P, FD], F32, tag="mm", name=f"ps_v{st}")
                    for ht in range(NH):
                        nc.tensor.matmul(
                            ps[:],
                            tokT[:, ht, st * P : (st + 1) * P],
                            wv16[:, ht, :],
                            start=(ht == 0),
                            stop=(ht == NH - 1),
                        )
                    nc.vector.tensor_copy(V[:, st, :], ps[:])

            # ---- scores S[s,t] + softmax over t (free axis) ----
            Etile = pp.tile([P, NT, T], F16, tag="E")
            for st in range(NT):
                pss = [
                    psp.tile([P, FD], F32, tag="mm", name=f"ps_s{st}_{tch}")
                    for tch in range(NC_T)
                ]
                for tch in range(NC_T):
                    for hb in range(NH):
                        nc.tensor.matmul(
                            pss[tch][:],
                            tokT[:, hb, st * P : (st + 1) * P],
                            GT[:, hb, tch * FD : (tch + 1) * FD],
                            start=(hb == 0),
                            stop=(hb == NH - 1),
                        )
                mx4 = stp.tile([P, NC_T], F32, tag="mx4")
                for tch in range(NC_T):
                    nc.vector.reduce_max(
                        mx4[:, tch : tch + 1], pss[tch][:], axis=AX.X
                    )
                nmx = stp.tile([P, 1], F32, tag="nmx")
                nc.vector.reduce_max(nmx[:], mx4[:], axis=AX.X, negate=True)
                racc = stp.tile([P, NC_T], F32, tag="racc")
                for tch in range(NC_T):
                    nc.scalar.activation(
                        Etile[:, st, tch * FD : (tch + 1) * FD],
                        pss[tch][:],
                        AF.Exp,
                        bias=nmx[:],
                        accum_out=racc[:, tch : tch + 1],
                    )
                rsum = stp.tile([P, 1], F32, tag="rsum")
                nc.vector.reduce_sum(rsum[:], racc[:], axis=AX.X)
                rinv = stp.tile([P, 1], F32, tag="rinv")
                nc.vector.reciprocal(rinv[:], rsum[:])
                # Fold 1/rowsum into V rows (rowsum is per-s, V is s-major).
                nc.vector.tensor_scalar_mul(V[:, st, :], V[:, st, :], rinv[:])

            # ---- context: ctx[t,h] = sum_s E[s,t] * V'[s,h] ----
            for tt in range(NT):
                ps = psp.tile([P, FD], F32, tag="mm", name=f"ps_c{tt}")
                for st in range(NT):
                    nc.tensor.matmul(
                        ps[:],
                        Etile[:, st, tt * P : (tt + 1) * P],
                        V[:, st, :],
                        start=(st == 0),
                        stop=(st == NT - 1),
                    )
                ot = osp.tile([P, H], F32, tag="ostage", name=f"ost{tt}")
                if tt >= NT - 2:
                    for cc in range(2):
                        sl = slice(cc * (H // 2), (cc + 1) * (H // 2))
                        nc.vector.tensor_copy(ot[:, sl], ps[:, sl])
                        nc.sync.dma_start(out_tiled[tt][:, sl], ot[:, sl])
                else:
                    nc.vector.tensor_copy(ot[:], ps[:])
                    nc.sync.dma_start(out_tiled[tt], ot[:])

    nc.compile()
    return nc


_NC = None


def _get_nc():
    global _NC
    if _NC is None:
        _NC = build()
    return _NC


def _run(inputs, trace=False, **kwargs):
    tokens = np.ascontiguousarray(inputs["tokens"], dtype=np.float32)
    Wq = np.ascontiguousarray(inputs["Wq"], dtype=np.float32)
    Wk = np.ascontiguousarray(inputs["Wk"], dtype=np.float32)
    Wv = np.ascontiguousarray(inputs["Wv"], dtype=np.float32)
    assert tokens.shape == (B, T, H)
    nc = _get_nc()
    in_maps = [
        {"tokens": tokens[i], "Wq": Wq, "Wk": Wk, "Wv": Wv} for i in range(N_CORES)
    ]
    res = run_bass_kernel_spmd(
        nc, in_maps, core_ids=list(range(N_CORES)), trace=trace, **kwargs
    )
    out = np.stack([res.results[i]["out"] for i in range(N_CORES)], axis=0)
    return out.astype(np.float32), res


def kernel(**inputs) -> np.ndarray:
    out, _ = _run(inputs)
    return out
